# revision 8
# baseline (speedup 1.0000x reference)
"""Bass/Trainium2 kernel for nn_KernelEdges (gnn_message_passing).

Computes A = exp((g_i + g_j - 2*Xf@Xf.T)/sigma^2) with zeroed diagonal,
broadcast to all B batch slots, where Xf = X.transpose(1,0,2).reshape(N, B*d).

Sharding: rows of the NxN pairwise matrix are split across 8 NeuronCores
(256 rows each).  Each core receives the full transposed operand
XT = Xf.T [B*d, N] (host-prepared, 4 MB), its own column-slice as the
stationary matmul operand, and writes its [B, N/8, N] output slice.

Per-core device work:
  psum[mt,nb] = sum_q XT_q[:, m_slice].T @ XT_q[:, n_block]     (Gram matrix)
              + (-1/2*ones).T @ g_row[n_block]                  (rank-1: -g_j/2)
  A = exp(-2/sigma^2 * psum + g_i/sigma^2)                      (ACT, bias per row)
  DMA A tile to the 8 batch slots of the output.

The diagonal is zeroed on the host (16K elements) after the gather.
"""

import numpy as np

B, N, D = 8, 2048, 64
NCORES = 8
R = N // NCORES          # 256 rows per core
KD = B * D               # 512 contraction dim
NB = 512                 # n-block (one PSUM bank of fp32)
NNB = N // NB            # 4 n-blocks
NMT = R // 128           # 2 m-tiles per core
NQ = KD // 128           # 4 k-tiles

# float32r: full-rate fp32 matmul mode (1 cycle/row at N>=256 vs 4 for fp32)
USE_F32R = True


def _build_program(inv_s2):
    import concourse.bass as bass
    import concourse.tile as tile
    from concourse import bacc, mybir

    f32 = mybir.dt.float32
    mm_dt = mybir.dt.float32r if USE_F32R else f32

    nc = bacc.Bacc(
        "TRN2", target_bir_lowering=False, debug=False, num_devices=NCORES
    )

    xt_d = nc.dram_tensor("xt", [KD, N], mm_dt, kind="ExternalInput").ap()
    lhst_d = nc.dram_tensor("lhst", [KD, R], mm_dt, kind="ExternalInput").ap()
    grow_d = nc.dram_tensor("grow", [1, N], mm_dt, kind="ExternalInput").ap()
    bias_d = nc.dram_tensor("bias", [128, NMT], f32, kind="ExternalInput").ap()
    out_d = nc.dram_tensor("out", [B, R, N], f32, kind="ExternalOutput").ap()

    with tile.TileContext(nc) as tc:
        with (
            tc.tile_pool(name="persist", bufs=1) as persist,
            tc.tile_pool(name="apool", bufs=1) as apool,
            tc.tile_pool(name="psum", bufs=1, space="PSUM") as pspool,
        ):
            # ---- loads ----
            # tiny tensors first: the rank-1 g_j matmuls depend only on
            # these, so they start during the xt load and warm the PE
            grow_sb = persist.tile([1, N], mm_dt, name="grow")
            nc.sync.dma_start(grow_sb[:], grow_d[:])

            bias_sb = persist.tile([128, NMT], f32, name="bias")
            nc.sync.dma_start(bias_sb[:], bias_d[:])

            neg_half = persist.tile([1, 128], mm_dt, name="neg_half")
            # -0.5f bit pattern; direct float memset into f32r fails ISA check
            nc.gpsimd.memset(neg_half[:].bitcast(mybir.dt.uint32), 0xBF000000)

            lhs_sb = persist.tile([128, NQ * R], mm_dt, name="lhs")
            nc.sync.dma_start(
                lhs_sb[:].rearrange("p (q m) -> p q m", q=NQ),
                lhst_d.rearrange("(q p) m -> p q m", p=128),
            )

            # xt tiles, each loaded as two half-DMAs so the trailing
            # k-tile's first half arrives sooner
            NH = N // 2
            xt_sb = []
            for q in range(NQ):
                t = persist.tile([128, N], mm_dt, name=f"xt{q}")
                for h in range(2):
                    nc.sync.dma_start(
                        t[:, h * NH:(h + 1) * NH],
                        xt_d[q * 128:(q + 1) * 128, h * NH:(h + 1) * NH],
                    )
                xt_sb.append(t)

            # ---- compute + store ----
            # all 8 accumulation chains live in the 8 PSUM banks at once;
            # chain order: rank-1 (g_j) first, then k-tiles q0..q3 as each
            # xt_q lands, so the PE overlaps the input DMA
            ps = {}
            for mt in range(NMT):
                for nb in range(NNB):
                    ps[mt, nb] = pspool.tile(
                        [128, NB], f32, name=f"ps{mt}{nb}"
                    )
                    nc.tensor.matmul(
                        ps[mt, nb][:],
                        neg_half[:],
                        grow_sb[0:1, nb * NB:(nb + 1) * NB],
                        start=True,
                        stop=False,
                    )
            for q in range(NQ):
                for h in range(2):
                    for mt in range(NMT):
                        m0 = q * R + mt * 128
                        for nb in range(2 * h, 2 * h + 2):
                            nc.tensor.matmul(
                                ps[mt, nb][:],
                                lhs_sb[:, m0:m0 + 128],
                                xt_sb[q][:, nb * NB:(nb + 1) * NB],
                                start=False,
                                stop=(q == NQ - 1),
                            )
                            if q == NQ - 1:
                                a_sb = apool.tile(
                                    [128, NB], f32, name=f"a{mt}{nb}"
                                )
                                nc.scalar.activation(
                                    a_sb[:],
                                    ps[mt, nb][:],
                                    mybir.ActivationFunctionType.Exp,
                                    bias=bias_sb[:, mt:mt + 1],
                                    scale=-2.0 * inv_s2,
                                )
                                # one DMA replicates the tile into all
                                # 8 batch slots of the output
                                src = a_sb[:].rearrange(
                                    "p (o n) -> p o n", o=1
                                ).broadcast_to([128, B, NB])
                                dst = out_d[
                                    :,
                                    mt * 128:(mt + 1) * 128,
                                    nb * NB:(nb + 1) * NB,
                                ].rearrange("b p n -> p b n")
                                nc.sync.dma_start(dst, src)

    nc.compile()
    return nc


def kernel(X, log_sigma):
    from concourse.bass_utils import run_bass_kernel_spmd

    X = np.ascontiguousarray(X, dtype=np.float32)
    assert X.shape == (B, N, D), X.shape

    sigma = float(np.exp(np.float32(log_sigma)))
    inv_s2 = 1.0 / (sigma * sigma)

    # XT[b*D+f, n] = X[b, n, f]
    XT = np.ascontiguousarray(X.transpose(0, 2, 1).reshape(KD, N))
    g = np.einsum("kn,kn->n", XT, XT).astype(np.float32)  # [N]

    nc = _build_program(inv_s2)

    in_maps = []
    for c in range(NCORES):
        r0 = c * R
        bias_np = np.empty((128, NMT), dtype=np.float32)
        for mt in range(NMT):
            bias_np[:, mt] = g[r0 + mt * 128: r0 + (mt + 1) * 128] * inv_s2
        in_maps.append({
            "xt": XT,
            "lhst": np.ascontiguousarray(XT[:, r0:r0 + R]),
            "grow": g[None, :],
            "bias": bias_np,
        })

    res = run_bass_kernel_spmd(nc, in_maps, list(range(NCORES)))
    out = np.concatenate([res.results[c]["out"] for c in range(NCORES)], axis=1)
    idx = np.arange(N)
    out[:, idx, idx] = 0.0
    return out


# revision 9
# speedup vs baseline: 1.1082x; 1.1082x over previous
"""Bass/Trainium2 kernel for nn_KernelEdges (gnn_message_passing).

Computes A = exp((g_i + g_j - 2*Xf@Xf.T)/sigma^2) with zeroed diagonal,
broadcast to all B batch slots, where Xf = X.transpose(1,0,2).reshape(N, B*d).

Sharding: rows of the NxN pairwise matrix are split across 8 NeuronCores
(256 rows each).  Each core receives the full transposed operand
XT = Xf.T [B*d, N] (host-prepared, 4 MB), its own column-slice as the
stationary matmul operand, and writes its [B, N/8, N] output slice.

Per-core device work:
  psum[mt,nb] = sum_q XT_q[:, m_slice].T @ XT_q[:, n_block]     (Gram matrix)
              + (-1/2*ones).T @ g_row[n_block]                  (rank-1: -g_j/2)
  A = exp(-2/sigma^2 * psum + g_i/sigma^2)                      (ACT, bias per row)
  DMA A tile to the 8 batch slots of the output.

The diagonal is zeroed on the host (16K elements) after the gather.
"""

import numpy as np

B, N, D = 8, 2048, 64
NCORES = 8
R = N // NCORES          # 256 rows per core
KD = B * D               # 512 contraction dim
NB = 512                 # n-block (one PSUM bank of fp32)
NNB = N // NB            # 4 n-blocks
NMT = R // 128           # 2 m-tiles per core
NQ = KD // 128           # 4 k-tiles

# float32r: full-rate fp32 matmul mode (1 cycle/row at N>=256 vs 4 for fp32)
USE_F32R = True


def _build_program(inv_s2):
    import concourse.bass as bass
    import concourse.tile as tile
    from concourse import bacc, mybir

    f32 = mybir.dt.float32
    mm_dt = mybir.dt.float32r if USE_F32R else f32

    nc = bacc.Bacc(
        "TRN2", target_bir_lowering=False, debug=False, num_devices=NCORES
    )

    xt_d = nc.dram_tensor("xt", [KD, N], mm_dt, kind="ExternalInput").ap()
    lhst_d = nc.dram_tensor("lhst", [KD, R], mm_dt, kind="ExternalInput").ap()
    grow_d = nc.dram_tensor("grow", [1, N], mm_dt, kind="ExternalInput").ap()
    bias_d = nc.dram_tensor("bias", [128, NMT], f32, kind="ExternalInput").ap()
    out_d = nc.dram_tensor("out", [B, R, N], f32, kind="ExternalOutput").ap()

    with tile.TileContext(nc) as tc:
        with (
            tc.tile_pool(name="persist", bufs=1) as persist,
            tc.tile_pool(name="apool", bufs=1) as apool,
            tc.tile_pool(name="psum", bufs=1, space="PSUM") as pspool,
        ):
            # ---- loads ----
            # all input DMAs go on the scalar (ACT) HWDGE ring so the sync
            # ring is dedicated to output DMAs.
            # tiny tensors first: the rank-1 g_j matmuls depend only on
            # these, so they start during the xt load and warm the PE
            grow_sb = persist.tile([1, N], mm_dt, name="grow")
            nc.scalar.dma_start(grow_sb[:], grow_d[:])

            bias_sb = persist.tile([128, NMT], f32, name="bias")
            nc.scalar.dma_start(bias_sb[:], bias_d[:])

            neg_half = persist.tile([1, 128], mm_dt, name="neg_half")
            # -0.5f bit pattern; direct float memset into f32r fails ISA check
            nc.gpsimd.memset(neg_half[:].bitcast(mybir.dt.uint32), 0xBF000000)

            lhs_sb = persist.tile([128, NQ * R], mm_dt, name="lhs")
            nc.scalar.dma_start(
                lhs_sb[:].rearrange("p (q m) -> p q m", q=NQ),
                lhst_d.rearrange("(q p) m -> p q m", p=128),
            )

            # xt tiles; the last one split in half so the trailing piece
            # (which gates the final matmul batch) is smaller
            NH = N // 2
            xt_sb = []
            for q in range(NQ):
                t = persist.tile([128, N], mm_dt, name=f"xt{q}")
                if q < NQ - 1:
                    nc.scalar.dma_start(t[:], xt_d[q * 128:(q + 1) * 128, :])
                else:
                    for h in range(2):
                        nc.scalar.dma_start(
                            t[:, h * NH:(h + 1) * NH],
                            xt_d[q * 128:(q + 1) * 128, h * NH:(h + 1) * NH],
                        )
                xt_sb.append(t)

            # ---- compute + store ----
            # all 8 accumulation chains live in the 8 PSUM banks at once;
            # chain order: rank-1 (g_j) first, then k-tiles q0..q3 as each
            # xt_q lands, so the PE overlaps the input DMA
            ps = {}
            for mt in range(NMT):
                for nb in range(NNB):
                    ps[mt, nb] = pspool.tile(
                        [128, NB], f32, name=f"ps{mt}{nb}"
                    )
                    nc.tensor.matmul(
                        ps[mt, nb][:],
                        neg_half[:],
                        grow_sb[0:1, nb * NB:(nb + 1) * NB],
                        start=True,
                        stop=False,
                    )
            for q in range(NQ):
                for h in range(2):
                    for mt in range(NMT):
                        m0 = q * R + mt * 128
                        for nb in range(2 * h, 2 * h + 2):
                            nc.tensor.matmul(
                                ps[mt, nb][:],
                                lhs_sb[:, m0:m0 + 128],
                                xt_sb[q][:, nb * NB:(nb + 1) * NB],
                                start=False,
                                stop=(q == NQ - 1),
                            )
                            if q == NQ - 1:
                                a_sb = apool.tile(
                                    [128, NB], f32, name=f"a{mt}{nb}"
                                )
                                nc.scalar.activation(
                                    a_sb[:],
                                    ps[mt, nb][:],
                                    mybir.ActivationFunctionType.Exp,
                                    bias=bias_sb[:, mt:mt + 1],
                                    scale=-2.0 * inv_s2,
                                )
                                # one DMA replicates the tile into all
                                # 8 batch slots of the output
                                src = a_sb[:].rearrange(
                                    "p (o n) -> p o n", o=1
                                ).broadcast_to([128, B, NB])
                                dst = out_d[
                                    :,
                                    mt * 128:(mt + 1) * 128,
                                    nb * NB:(nb + 1) * NB,
                                ].rearrange("b p n -> p b n")
                                nc.sync.dma_start(dst, src)

    nc.compile()
    return nc


def kernel(X, log_sigma):
    from concourse.bass_utils import run_bass_kernel_spmd

    X = np.ascontiguousarray(X, dtype=np.float32)
    assert X.shape == (B, N, D), X.shape

    sigma = float(np.exp(np.float32(log_sigma)))
    inv_s2 = 1.0 / (sigma * sigma)

    # XT[b*D+f, n] = X[b, n, f]
    XT = np.ascontiguousarray(X.transpose(0, 2, 1).reshape(KD, N))
    g = np.einsum("kn,kn->n", XT, XT).astype(np.float32)  # [N]

    nc = _build_program(inv_s2)

    in_maps = []
    for c in range(NCORES):
        r0 = c * R
        bias_np = np.empty((128, NMT), dtype=np.float32)
        for mt in range(NMT):
            bias_np[:, mt] = g[r0 + mt * 128: r0 + (mt + 1) * 128] * inv_s2
        in_maps.append({
            "xt": XT,
            "lhst": np.ascontiguousarray(XT[:, r0:r0 + R]),
            "grow": g[None, :],
            "bias": bias_np,
        })

    res = run_bass_kernel_spmd(nc, in_maps, list(range(NCORES)))
    out = np.concatenate([res.results[c]["out"] for c in range(NCORES)], axis=1)
    idx = np.arange(N)
    out[:, idx, idx] = 0.0
    return out
